# revision 1
# baseline (speedup 1.0000x reference)
"""Trainium2 Bass kernel for nn_EntityMapping (P=16 independent MLPs over a
shared entity batch).

Sharding: the 16 partition-MLPs are split across 8 NeuronCores (2 per core,
expert-parallel); the embedding batch is replicated. Activations are kept
feature-major [feature, batch] on-chip so every layer is a chain of
128x128-stationary matmuls with the batch streaming through the PE array.
Matmuls run in float32r (full-rate fp32 on TRN2's PE at N>=256; inputs are
rounded to fp32r by DVE/ACT producer ops as walrus requires).
"""

import numpy as np

try:
    import concourse.bass as bass  # noqa: F401
except ImportError:  # harness runs kernel.py from a bare directory
    import sys

    sys.path.insert(0, "/opt/trn_rl_repo")

import concourse.mybir as mybir
import concourse.tile as tile
from concourse import bacc
from concourse.bass_utils import run_bass_kernel_spmd

F32 = mybir.dt.float32
F32R = mybir.dt.float32r
RELU = mybir.ActivationFunctionType.Relu
SIGMOID = mybir.ActivationFunctionType.Sigmoid

P_TOTAL = 16  # independent MLP partitions
E = 512  # entity/embedding dim
H = 512  # hidden dim
N = 8192  # batch (entities)
N_CORES = 8
P_PER = P_TOTAL // N_CORES  # 2 MLPs per core
KC = E // 128  # 4 contraction chunks per layer
JC = H // 128  # 4 output-feature chunks per layer
NCH = 512  # batch columns per n-chunk (= fp32 moving-operand max = PSUM bank)
NCHUNKS = N // NCH  # 16

# ACT (scalar engine) writing float32r directly is accepted by walrus as an
# fp32r rounding producer; if False, relu outputs staging f32 tiles that DVE
# round-copies to f32r.
ACT_WRITES_F32R = True


def _widx(p, k, j):
    return ((p * KC + k) * JC + j) * 128


def _build():
    nc = bacc.Bacc(
        "TRN2", target_bir_lowering=False, debug=False, num_devices=N_CORES
    )
    eT_dram = nc.dram_tensor("eT", [E, N], F32, kind="ExternalInput")
    w0_dram = nc.dram_tensor("w0", [P_PER, KC, JC, 128, 128], F32, kind="ExternalInput")
    b0_dram = nc.dram_tensor("b0", [P_PER, JC, 128], F32, kind="ExternalInput")
    w1_dram = nc.dram_tensor("w1", [P_PER, KC, JC, 128, 128], F32, kind="ExternalInput")
    b1_dram = nc.dram_tensor("b1", [P_PER, JC, 128], F32, kind="ExternalInput")
    w2_dram = nc.dram_tensor("w2", [P_PER, KC, 128], F32, kind="ExternalInput")
    b2_dram = nc.dram_tensor("b2", [1, P_PER], F32, kind="ExternalInput")
    out_dram = nc.dram_tensor("out", [P_PER, N], F32, kind="ExternalOutput")

    with tile.TileContext(nc) as tc:
        with (
            tc.tile_pool(name="wconst", bufs=1) as wconst,
            tc.tile_pool(name="wstage", bufs=1) as wstage,
            tc.tile_pool(name="et", bufs=2) as et_pool,
            tc.tile_pool(name="etr", bufs=2) as etr_pool,
            tc.tile_pool(name="act", bufs=2) as act_pool,
            tc.tile_pool(name="hstage", bufs=3) as hstage_pool,
            tc.tile_pool(name="osb", bufs=4) as out_pool,
            tc.tile_pool(name="mmps", bufs=6, space="PSUM") as ps_mm,
            tc.tile_pool(name="l2ps", bufs=2, space="PSUM") as ps_l2,
        ):
            # --- persistent weights/biases, rounded to f32r once ---
            NW = P_PER * KC * JC * 128  # 4096 cols
            w_stage = wstage.tile([128, NW], F32, tag="wstage")
            w0_r = wconst.tile([128, NW], F32R, tag="w0r")
            w1_r = wconst.tile([128, NW], F32R, tag="w1r")
            for wi, (wd, wr) in enumerate([(w0_dram, w0_r), (w1_dram, w1_r)]):
                for p in range(P_PER):
                    for k in range(KC):
                        for j in range(JC):
                            off = _widx(p, k, j)
                            nc.sync.dma_start(
                                w_stage[:, off : off + 128], wd[p, k, j]
                            )
                nc.vector.tensor_copy(wr[:], w_stage[:])

            w2_stage = wstage.tile([128, P_PER * KC], F32, tag="w2stage")
            w2_r = wconst.tile([128, P_PER * KC], F32R, tag="w2r")
            for p in range(P_PER):
                for k in range(KC):
                    nc.sync.dma_start(
                        w2_stage[:, p * KC + k : p * KC + k + 1], w2_dram[p, k][:, None]
                    )
            nc.vector.tensor_copy(w2_r[:], w2_stage[:])

            b0_sb = wconst.tile([128, P_PER * JC], F32, tag="b0")
            b1_sb = wconst.tile([128, P_PER * JC], F32, tag="b1")
            for p in range(P_PER):
                for j in range(JC):
                    nc.sync.dma_start(
                        b0_sb[:, p * JC + j : p * JC + j + 1], b0_dram[p, j][:, None]
                    )
                    nc.sync.dma_start(
                        b1_sb[:, p * JC + j : p * JC + j + 1], b1_dram[p, j][:, None]
                    )
            b2_sb = wconst.tile([1, P_PER], F32, tag="b2")
            nc.sync.dma_start(b2_sb[:], b2_dram[:])

            # --- main loop over batch chunks ---
            for c in range(NCHUNKS):
                n0 = c * NCH
                et_f = et_pool.tile([128, KC * NCH], F32, tag="et")
                et = etr_pool.tile([128, KC * NCH], F32R, tag="etr")
                for k in range(KC):
                    nc.sync.dma_start(
                        et_f[:, k * NCH : (k + 1) * NCH],
                        eT_dram[k * 128 : (k + 1) * 128, n0 : n0 + NCH],
                    )
                    nc.vector.tensor_copy(
                        et[:, k * NCH : (k + 1) * NCH],
                        et_f[:, k * NCH : (k + 1) * NCH],
                    )

                for p in range(P_PER):
                    # L0: h = relu(W0^T eT + b0), feature-major [H, NCH]
                    h = act_pool.tile([128, JC * NCH], F32R, tag="h")
                    for j in range(JC):
                        ps = ps_mm.tile([128, NCH], F32, tag="mm")
                        for k in range(KC):
                            off = _widx(p, k, j)
                            nc.tensor.matmul(
                                ps[:],
                                w0_r[:, off : off + 128],
                                et[:, k * NCH : (k + 1) * NCH],
                                start=(k == 0),
                                stop=(k == KC - 1),
                            )
                        bia = b0_sb[:, p * JC + j : p * JC + j + 1]
                        if ACT_WRITES_F32R:
                            nc.scalar.activation(
                                h[:, j * NCH : (j + 1) * NCH], ps[:], RELU, bias=bia
                            )
                        else:
                            hs = hstage_pool.tile([128, NCH], F32, tag="hs")
                            nc.scalar.activation(hs[:], ps[:], RELU, bias=bia)
                            nc.vector.tensor_copy(
                                h[:, j * NCH : (j + 1) * NCH], hs[:]
                            )

                    # L1: h2 = relu(W1^T h + b1)
                    h2 = act_pool.tile([128, JC * NCH], F32R, tag="h2")
                    for j in range(JC):
                        ps = ps_mm.tile([128, NCH], F32, tag="mm")
                        for k in range(KC):
                            off = _widx(p, k, j)
                            nc.tensor.matmul(
                                ps[:],
                                w1_r[:, off : off + 128],
                                h[:, k * NCH : (k + 1) * NCH],
                                start=(k == 0),
                                stop=(k == KC - 1),
                            )
                        bia = b1_sb[:, p * JC + j : p * JC + j + 1]
                        if ACT_WRITES_F32R:
                            nc.scalar.activation(
                                h2[:, j * NCH : (j + 1) * NCH], ps[:], RELU, bias=bia
                            )
                        else:
                            hs = hstage_pool.tile([128, NCH], F32, tag="hs")
                            nc.scalar.activation(hs[:], ps[:], RELU, bias=bia)
                            nc.vector.tensor_copy(
                                h2[:, j * NCH : (j + 1) * NCH], hs[:]
                            )

                    # L2: out = sigmoid(W2^T h2 + b2), [1, NCH]
                    r = ps_l2.tile([1, NCH], F32, tag="l2")
                    for k in range(KC):
                        nc.tensor.matmul(
                            r[:],
                            w2_r[:, p * KC + k : p * KC + k + 1],
                            h2[:, k * NCH : (k + 1) * NCH],
                            start=(k == 0),
                            stop=(k == KC - 1),
                        )
                    o = out_pool.tile([1, NCH], F32, tag="o")
                    nc.scalar.activation(
                        o[:], r[:], SIGMOID, bias=b2_sb[0:1, p : p + 1]
                    )
                    nc.sync.dma_start(out_dram[p : p + 1, n0 : n0 + NCH], o[:])

    nc.compile()
    return nc


_NC_CACHE = None


def _get_nc():
    global _NC_CACHE
    if _NC_CACHE is None:
        _NC_CACHE = _build()
    return _NC_CACHE


def _make_in_maps(e_embedding, W0, b0, W1, b1, W2, b2):
    e = np.asarray(e_embedding, dtype=np.float32)
    W0 = np.asarray(W0, dtype=np.float32)
    b0 = np.asarray(b0, dtype=np.float32)
    W1 = np.asarray(W1, dtype=np.float32)
    b1 = np.asarray(b1, dtype=np.float32)
    W2 = np.asarray(W2, dtype=np.float32)
    b2 = np.asarray(b2, dtype=np.float32)

    eT = np.ascontiguousarray(e.T)  # [E, N]
    in_maps = []
    for cid in range(N_CORES):
        sl = slice(P_PER * cid, P_PER * (cid + 1))
        w0t = np.ascontiguousarray(
            W0[sl].reshape(P_PER, KC, 128, JC, 128).transpose(0, 1, 3, 2, 4)
        )
        w1t = np.ascontiguousarray(
            W1[sl].reshape(P_PER, KC, 128, JC, 128).transpose(0, 1, 3, 2, 4)
        )
        in_maps.append(
            {
                "eT": eT,
                "w0": w0t,
                "b0": np.ascontiguousarray(b0[sl].reshape(P_PER, JC, 128)),
                "w1": w1t,
                "b1": np.ascontiguousarray(b1[sl].reshape(P_PER, JC, 128)),
                "w2": np.ascontiguousarray(W2[sl, :, 0].reshape(P_PER, KC, 128)),
                "b2": np.ascontiguousarray(b2[sl].reshape(1, P_PER)),
            }
        )
    return in_maps


def kernel_with_results(trace=False, **inputs):
    nc = _get_nc()
    in_maps = _make_in_maps(**inputs)
    res = run_bass_kernel_spmd(
        nc, in_maps, core_ids=list(range(N_CORES)), trace=trace
    )
    full = np.concatenate([r["out"] for r in res.results], axis=0)  # [16, N]
    out = np.ascontiguousarray(full.T).astype(np.float32)  # [N, 16]
    return out, res


def kernel(**inputs):
    out, _ = kernel_with_results(trace=False, **inputs)
    return out


# revision 2
# speedup vs baseline: 1.1271x; 1.1271x over previous
"""Trainium2 Bass kernel for nn_EntityMapping (P=16 independent MLPs over a
shared entity batch).

Sharding: the 16 partition-MLPs are split across 8 NeuronCores (2 per core,
expert-parallel); the embedding batch is replicated. Activations are kept
feature-major [feature, batch] on-chip so every layer is a chain of
128x128-stationary matmuls with the batch streaming through the PE array.
Matmuls run in float32r (full-rate fp32 on TRN2's PE at N>=256; inputs are
rounded to fp32r by DVE/ACT producer ops as walrus requires).

Host-side prep packs every weight/bias into the exact SBUF layout so each
lands with a single contiguous DMA; embedding chunks stream on the gpsimd
queue while weights use sync, so the first matmul issues ~10us in.
"""

import numpy as np

try:
    import concourse.bass as bass  # noqa: F401
except ImportError:  # harness runs kernel.py from a bare directory
    import sys

    sys.path.insert(0, "/opt/trn_rl_repo")

import concourse.mybir as mybir
import concourse.tile as tile
from concourse import bacc
from concourse.bass_utils import run_bass_kernel_spmd

F32 = mybir.dt.float32
F32R = mybir.dt.float32r
RELU = mybir.ActivationFunctionType.Relu
SIGMOID = mybir.ActivationFunctionType.Sigmoid
COPY = mybir.ActivationFunctionType.Copy

P_TOTAL = 16  # independent MLP partitions
E = 512  # entity/embedding dim
H = 512  # hidden dim
N = 8192  # batch (entities)
N_CORES = 8
P_PER = P_TOTAL // N_CORES  # 2 MLPs per core
KC = E // 128  # 4 contraction chunks per layer
JC = H // 128  # 4 output-feature chunks per layer
NCH = 512  # batch columns per n-chunk (= fp32 moving-operand max = PSUM bank)
NCHUNKS = N // NCH  # 16
NW = P_PER * KC * JC  # 32 weight tiles per layer


def _build():
    nc = bacc.Bacc(
        "TRN2", target_bir_lowering=False, debug=False, num_devices=N_CORES
    )
    # All inputs pre-packed on host into SBUF-layout [128, cols]:
    eT_dram = nc.dram_tensor("eT", [E, N], F32, kind="ExternalInput")
    w0_dram = nc.dram_tensor("w0", [128, NW * 128], F32, kind="ExternalInput")
    w1_dram = nc.dram_tensor("w1", [128, NW * 128], F32, kind="ExternalInput")
    b0_dram = nc.dram_tensor("b0", [128, P_PER * JC], F32, kind="ExternalInput")
    b1_dram = nc.dram_tensor("b1", [128, P_PER * JC], F32, kind="ExternalInput")
    w2_dram = nc.dram_tensor("w2", [128, P_PER * KC], F32, kind="ExternalInput")
    b2_dram = nc.dram_tensor("b2", [1, P_PER], F32, kind="ExternalInput")
    out_dram = nc.dram_tensor("out", [P_PER, N], F32, kind="ExternalOutput")

    # eT viewed as [ki, k, n] for single-instruction chunk DMAs
    eT_v = eT_dram.rearrange("(k ki) n -> ki k n", ki=128)

    with tile.TileContext(nc) as tc:
        with (
            tc.tile_pool(name="wconst", bufs=1) as wconst,
            tc.tile_pool(name="wstage", bufs=2) as wstage,
            tc.tile_pool(name="et", bufs=2) as et_pool,
            tc.tile_pool(name="etr", bufs=2) as etr_pool,
            tc.tile_pool(name="act", bufs=2) as act_pool,
            tc.tile_pool(name="osb", bufs=4) as out_pool,
            tc.tile_pool(name="mmps", bufs=6, space="PSUM") as ps_mm,
            tc.tile_pool(name="l2ps", bufs=2, space="PSUM") as ps_l2,
        ):
            # --- persistent weights/biases, single-DMA loads, rounded to
            # f32r once.  w0 rounds on DVE (needed first), w1 on ACT so the
            # DVE queue is free for the first embedding-chunk casts. ---
            w0_stage = wstage.tile([128, NW * 128], F32, tag="wstage")
            nc.sync.dma_start(w0_stage[:], w0_dram[:])
            w0_r = wconst.tile([128, NW, 128], F32R, tag="w0r")
            nc.vector.tensor_copy(
                w0_r[:].rearrange("p a b -> p (a b)"), w0_stage[:]
            )

            w1_stage = wstage.tile([128, NW * 128], F32, tag="wstage")
            nc.sync.dma_start(w1_stage[:], w1_dram[:])
            w1_r = wconst.tile([128, NW, 128], F32R, tag="w1r")
            nc.scalar.activation(
                w1_r[:].rearrange("p a b -> p (a b)"), w1_stage[:], COPY
            )

            w2_stage = wstage.tile([128, P_PER * KC], F32, tag="w2stage")
            nc.sync.dma_start(w2_stage[:], w2_dram[:])
            w2_r = wconst.tile([128, P_PER * KC], F32R, tag="w2r")
            nc.scalar.activation(w2_r[:], w2_stage[:], COPY)

            b0_sb = wconst.tile([128, P_PER * JC], F32, tag="b0")
            nc.sync.dma_start(b0_sb[:], b0_dram[:])
            b1_sb = wconst.tile([128, P_PER * JC], F32, tag="b1")
            nc.sync.dma_start(b1_sb[:], b1_dram[:])
            b2_sb = wconst.tile([1, P_PER], F32, tag="b2")
            nc.sync.dma_start(b2_sb[:], b2_dram[:])

            # --- main loop over batch chunks ---
            for c in range(NCHUNKS):
                n0 = c * NCH
                et_f = et_pool.tile([128, KC, NCH], F32, tag="et")
                nc.gpsimd.dma_start(et_f[:], eT_v[:, :, n0 : n0 + NCH])
                et = etr_pool.tile([128, KC, NCH], F32R, tag="etr")
                for k in range(KC):
                    nc.vector.tensor_copy(et[:, k, :], et_f[:, k, :])

                for p in range(P_PER):
                    # L0: h = relu(W0^T eT + b0), feature-major [H, NCH]
                    h = act_pool.tile([128, JC, NCH], F32R, tag="h")
                    for j in range(JC):
                        ps = ps_mm.tile([128, NCH], F32, tag="mm")
                        for k in range(KC):
                            wi = (p * KC + k) * JC + j
                            nc.tensor.matmul(
                                ps[:],
                                w0_r[:, wi, :],
                                et[:, k, :],
                                start=(k == 0),
                                stop=(k == KC - 1),
                            )
                        nc.scalar.activation(
                            h[:, j, :],
                            ps[:],
                            RELU,
                            bias=b0_sb[:, p * JC + j : p * JC + j + 1],
                        )

                    # L1: h2 = relu(W1^T h + b1)
                    h2 = act_pool.tile([128, JC, NCH], F32R, tag="h2")
                    for j in range(JC):
                        ps = ps_mm.tile([128, NCH], F32, tag="mm")
                        for k in range(KC):
                            wi = (p * KC + k) * JC + j
                            nc.tensor.matmul(
                                ps[:],
                                w1_r[:, wi, :],
                                h[:, k, :],
                                start=(k == 0),
                                stop=(k == KC - 1),
                            )
                        nc.scalar.activation(
                            h2[:, j, :],
                            ps[:],
                            RELU,
                            bias=b1_sb[:, p * JC + j : p * JC + j + 1],
                        )

                    # L2: out = sigmoid(W2^T h2 + b2), [1, NCH]
                    r = ps_l2.tile([1, NCH], F32, tag="l2")
                    for k in range(KC):
                        nc.tensor.matmul(
                            r[:],
                            w2_r[:, p * KC + k : p * KC + k + 1],
                            h2[:, k, :],
                            start=(k == 0),
                            stop=(k == KC - 1),
                        )
                    o = out_pool.tile([1, NCH], F32, tag="o")
                    nc.scalar.activation(
                        o[:], r[:], SIGMOID, bias=b2_sb[0:1, p : p + 1]
                    )
                    nc.sync.dma_start(out_dram[p : p + 1, n0 : n0 + NCH], o[:])

    nc.compile()
    return nc


_NC_CACHE = None


def _get_nc():
    global _NC_CACHE
    if _NC_CACHE is None:
        _NC_CACHE = _build()
    return _NC_CACHE


def _make_in_maps(e_embedding, W0, b0, W1, b1, W2, b2):
    e = np.asarray(e_embedding, dtype=np.float32)
    W0 = np.asarray(W0, dtype=np.float32)
    b0 = np.asarray(b0, dtype=np.float32)
    W1 = np.asarray(W1, dtype=np.float32)
    b1 = np.asarray(b1, dtype=np.float32)
    W2 = np.asarray(W2, dtype=np.float32)
    b2 = np.asarray(b2, dtype=np.float32)

    eT = np.ascontiguousarray(e.T)  # [E, N]
    in_maps = []
    for cid in range(N_CORES):
        sl = slice(P_PER * cid, P_PER * (cid + 1))
        # SBUF layout [ki, ((p k j) ji)]: row ki, tile (p,k,j), col ji
        w0t = np.ascontiguousarray(
            W0[sl]
            .reshape(P_PER, KC, 128, JC, 128)
            .transpose(2, 0, 1, 3, 4)
            .reshape(128, NW * 128)
        )
        w1t = np.ascontiguousarray(
            W1[sl]
            .reshape(P_PER, KC, 128, JC, 128)
            .transpose(2, 0, 1, 3, 4)
            .reshape(128, NW * 128)
        )
        # [ki, (p j)] bias columns
        b0t = np.ascontiguousarray(
            b0[sl].reshape(P_PER, JC, 128).transpose(2, 0, 1).reshape(128, -1)
        )
        b1t = np.ascontiguousarray(
            b1[sl].reshape(P_PER, JC, 128).transpose(2, 0, 1).reshape(128, -1)
        )
        w2t = np.ascontiguousarray(
            W2[sl, :, 0].reshape(P_PER, KC, 128).transpose(2, 0, 1).reshape(128, -1)
        )
        b2t = np.ascontiguousarray(b2[sl].reshape(1, P_PER))
        in_maps.append(
            {"eT": eT, "w0": w0t, "b0": b0t, "w1": w1t, "b1": b1t,
             "w2": w2t, "b2": b2t}
        )
    return in_maps


def kernel_with_results(trace=False, **inputs):
    nc = _get_nc()
    in_maps = _make_in_maps(**inputs)
    res = run_bass_kernel_spmd(
        nc, in_maps, core_ids=list(range(N_CORES)), trace=trace
    )
    full = np.concatenate([r["out"] for r in res.results], axis=0)  # [16, N]
    out = np.ascontiguousarray(full.T).astype(np.float32)  # [N, 16]
    return out, res


def kernel(**inputs):
    out, _ = kernel_with_results(trace=False, **inputs)
    return out
